# revision 7
# baseline (speedup 1.0000x reference)
"""Trainium2 Bass kernel for CoreAttentionExpand (sparse local+global attention).

Sharding: tensor-parallel over heads. 16 heads / 8 cores = 2 heads per core.
Each core computes RoPE + local-block attention + pooled-global attention for
its 2 heads end-to-end (no collectives); host reassembles the full output.

Device layout: head-dim-major [D=128 partitions, T] for q/k so QK^T and PV
matmuls contract over partitions. Scores are computed transposed
(S^T = K @ Q^T -> [k-tokens, q-tokens]) so exp(S^T) feeds the PV matmul
directly as the moving operand (no P transposes). Causal/history/global
masks are applied by accumulating -1e4 ramp matmuls (bf16) into the score
PSUM before exp; exp underflows those entries to exactly 0.
"""

import sys
import math

_REPO = "/opt/trn_rl_repo"
if _REPO not in sys.path:
    sys.path.insert(0, _REPO)

import numpy as np
import ml_dtypes

# ---------------------------------------------------------------- constants
H = 16          # heads
D = 128         # head dim
T = 4096        # tokens
L = 1024        # local block size
S = 128         # global pool stride
E = 128         # local history size
PNUM = T // L   # 4 local blocks
KLEN = T // S + 1  # 33 global keys (incl. zero token)
NCORES = 8
HPC = H // NCORES  # 2 heads per core
NEGBIG = -10000.0
SCALE = 1.0 / math.sqrt(D)
CHUNK = 512     # q-chunk width

_CACHE = {}


def _apply_framework_patches(bassmod, mybir, tilemod):
    """This walrus build rejects >1 sem wait per instruction; split excess
    waits onto preceding same-engine NoOps (pure scheduling transform)."""
    if getattr(tilemod.TileContext, "_wait_split_patched", False):
        return
    TileContext = tilemod.TileContext
    ScopedClock = tilemod.ScopedClock

    orig_add = TileContext._add_instruction
    ctr = [0]

    def split_add(self, inst):
        si = inst.sync_info
        if si is not None and si.on_wait and len(si.on_wait) > 1:
            ow = list(si.on_wait)
            for w in ow[:-1]:
                ctr[0] += 1
                nop = mybir.InstNoOp(name=f"I-wsplit{ctr[0]}", engine=inst.engine)
                nop.sync_info = mybir.SyncInfo(on_wait=[w], on_update=[])
                orig_add(self, nop)
            si.on_wait = [ow[-1]]
        orig_add(self, inst)

    def drain_and_barrier(self, tick_clock, wait_clock):
        nc = self.nc
        probe = nc.sync.nop(nofuse=True, hint="waitprobe")
        wait_clock.add_sem_waits(
            probe.ins, ScopedClock({None: tick_clock.global_clock})
        )
        si = probe.ins.sync_info
        ow = list(si.on_wait) if si and si.on_wait else []
        if len(ow) > 1:
            si.on_wait = ow[:1]
            for w in ow[1:]:
                n2 = nc.sync.nop(nofuse=True, hint="waitsplit")
                n2.ins.sync_info = mybir.SyncInfo(on_wait=[w], on_update=[])
        nc.sync.drain()
        nc.all_engine_barrier()
        popped = nc._tile_sem_poison_stack.pop()
        assert popped is self._sem_poison
        nc.clear_and_free_semaphores(list(self.sems.allocated().values()))
        nc.all_engine_barrier()

    TileContext._add_instruction = split_add
    TileContext._drain_and_barrier = drain_and_barrier
    TileContext._wait_split_patched = True


# ---------------------------------------------------------------- constants (host)
def _host_constants():
    t = np.arange(T, dtype=np.float32)
    inv = (1.0 / (10000.0 ** (np.arange(0, D, 2, dtype=np.float32) / D))).astype(
        np.float32
    )  # [64]
    emb = t[:, None] * inv[None, :]          # [T, 64]
    cos64 = np.cos(emb).astype(np.float32)
    sin64 = np.sin(emb).astype(np.float32)
    # [D, T] head-dim-major tables
    cosT = np.ascontiguousarray(np.concatenate([cos64, cos64], axis=1).T)
    sinRT = np.ascontiguousarray(np.concatenate([-sin64, sin64], axis=1).T)

    bf = ml_dtypes.bfloat16
    idx = np.arange(128)
    # causal ramp: (mB^T mC)[k, q] = NEGBIG * max(k - q, 0)
    mB = (idx[:, None] <= idx[None, :]).astype(bf)              # [m, k]: m <= k
    mC = (NEGBIG * (idx[:, None] > idx[None, :])).astype(bf)    # [m, q]: m > q
    ones_bf = np.ones((128, 128), dtype=bf)
    # global stairstep: for chunk c, row j, masked cols qq < 128*(j - 4c)
    gB = np.zeros((3, 8 * KLEN), dtype=np.float32)
    for c in range(8):
        for mm in range(3):
            for j in range(KLEN):
                gB[mm, KLEN * c + j] = 1.0 if j >= 4 * c + mm + 1 else 0.0
    gB = gB.astype(bf)
    qq = np.arange(CHUNK)
    gC = np.stack(
        [NEGBIG * ((qq >= 128 * mm) & (qq < 128 * (mm + 1))) for mm in range(3)]
    ).astype(bf)                                                # [3, 512]
    ident = np.eye(128, dtype=np.float32)
    poolcol = np.full((128, 1), 1.0 / S, dtype=bf)              # pooling matmul rhs
    return {
        "cosT": cosT,
        "sinRT": sinRT,
        "mB": mB,
        "mC": mC,
        "ones_bf": ones_bf,
        "gB": gB,
        "gC": gC,
        "ident": ident,
        "poolcol": poolcol,
    }


# ---------------------------------------------------------------- device program
def _build_program():
    import concourse.bass as bass
    import concourse.mybir as mybir
    import concourse.tile as tile

    _apply_framework_patches(bass, mybir, tile)

    f32 = mybir.dt.float32
    f32r = mybir.dt.float32r
    bf16 = mybir.dt.bfloat16
    EXP = mybir.ActivationFunctionType.Exp
    MUL = mybir.AluOpType.mult
    ADD = mybir.AluOpType.add

    nc = bass.Bass()
    qT_d = nc.dram_tensor("qT", [HPC, D, T], f32, kind="ExternalInput")
    kT_d = nc.dram_tensor("kT", [HPC, D, T], f32, kind="ExternalInput")
    v_d = nc.dram_tensor("v", [HPC, T, D], f32, kind="ExternalInput")
    zk_d = nc.dram_tensor("zk", [HPC, D, 1], f32, kind="ExternalInput")
    zv_d = nc.dram_tensor("zv", [HPC, D, 1], f32, kind="ExternalInput")
    cosT_d = nc.dram_tensor("cosT", [D, T], f32, kind="ExternalInput")
    sinRT_d = nc.dram_tensor("sinRT", [D, T], f32, kind="ExternalInput")
    mB_d = nc.dram_tensor("mB", [128, 128], bf16, kind="ExternalInput")
    mC_d = nc.dram_tensor("mC", [128, 128], bf16, kind="ExternalInput")
    ones_d = nc.dram_tensor("ones_bf", [128, 128], bf16, kind="ExternalInput")
    gB_d = nc.dram_tensor("gB", [3, 8 * KLEN], bf16, kind="ExternalInput")
    gC_d = nc.dram_tensor("gC", [3, CHUNK], bf16, kind="ExternalInput")
    ident_d = nc.dram_tensor("ident", [128, 128], f32, kind="ExternalInput")
    poolcol_d = nc.dram_tensor("poolcol", [128, 1], bf16, kind="ExternalInput")
    out_d = nc.dram_tensor("outT", [HPC, D, T], f32, kind="ExternalOutput")

    NT = T // 128  # 32 token-tiles per head

    with tile.TileContext(nc) as tc:
        with (
            tc.tile_pool(name="persist", bufs=1) as persist,
            tc.tile_pool(name="vload", bufs=1) as vload,
            tc.tile_pool(name="expp", bufs=12) as expp,
            tc.tile_pool(name="small", bufs=4) as small,
            tc.tile_pool(name="combine", bufs=2) as combine,
            tc.tile_pool(name="scores", bufs=2, space="PSUM") as scores_p,
            tc.tile_pool(name="acc", bufs=1, space="PSUM") as acc_p,
        ):
            # ---- small constants (live whole kernel)
            mB = persist.tile([128, 128], bf16, tag="mB")
            mC = persist.tile([128, 128], bf16, tag="mC")
            ones_bf = persist.tile([128, 128], bf16, tag="ones")
            gB = persist.tile([3, 8 * KLEN], bf16, tag="gB")
            gC = persist.tile([3, CHUNK], bf16, tag="gC")
            ident = persist.tile([128, 128], f32, tag="ident")
            poolcol = persist.tile([128, 1], bf16, tag="poolcol")
            nc.sync.dma_start(out=mB, in_=mB_d.ap())
            nc.sync.dma_start(out=mC, in_=mC_d.ap())
            nc.sync.dma_start(out=ones_bf, in_=ones_d.ap())
            nc.sync.dma_start(out=gB, in_=gB_d.ap())
            nc.sync.dma_start(out=gC, in_=gC_d.ap())
            nc.sync.dma_start(out=ident, in_=ident_d.ap())
            nc.sync.dma_start(out=poolcol, in_=poolcol_d.ap())

            QT, KT = {}, {}
            # ================= phase 1: RoPE (tables + transients freed after)
            with (
                tc.tile_pool(name="tables", bufs=1) as tables,
                tc.tile_pool(name="rope", bufs=2) as rope,
            ):
                cosT = tables.tile([D, T], f32, tag="cosT")
                sinRT = tables.tile([D, T], f32, tag="sinRT")
                nc.sync.dma_start(out=cosT, in_=cosT_d.ap())
                nc.sync.dma_start(out=sinRT, in_=sinRT_d.ap())
                for h in range(HPC):
                    QT[h] = persist.tile([D, T], f32r, tag=f"QT{h}", name=f"QT{h}")
                    KT[h] = persist.tile([D, T], f32r, tag=f"KT{h}", name=f"KT{h}")
                    for src_d, dst in ((qT_d, QT[h]), (kT_d, KT[h])):
                        for c0 in range(0, T, 1024):
                            cs = slice(c0, c0 + 1024)
                            raw = rope.tile([D, 1024], f32, tag="raw")
                            rot = rope.tile([D, 1024], f32, tag="rot")
                            tmp = rope.tile([D, 1024], f32, tag="tmp")
                            tmp2 = rope.tile([D, 1024], f32, tag="tmp2", bufs=1)
                            nc.sync.dma_start(out=raw, in_=src_d.ap()[h, :, cs])
                            nc.sync.dma_start(
                                out=rot[0:64, :], in_=src_d.ap()[h, 64:128, cs]
                            )
                            nc.sync.dma_start(
                                out=rot[64:128, :], in_=src_d.ap()[h, 0:64, cs]
                            )
                            # tmp = rot * sinRT  (GPSIMD, SBUF-only)
                            nc.gpsimd.tensor_tensor(
                                out=tmp, in0=rot, in1=sinRT[:, cs], op=MUL
                            )
                            nc.vector.tensor_tensor(
                                out=tmp2, in0=raw, in1=cosT[:, cs], op=MUL
                            )
                            nc.vector.tensor_tensor(
                                out=dst[:, cs], in0=tmp2, in1=tmp, op=ADD
                            )

            # ================= phase 2: attention per head
            for h in range(HPC):
                # V load + bf16 cast (token-major tiles [t%128, tile, d])
                vraw = vload.tile([128, NT, D], f32, tag="vraw")
                nc.sync.dma_start(
                    out=vraw, in_=v_d.ap()[h].rearrange("(n p) d -> p n d", p=128)
                )
                vbf = persist.tile([128, NT, D], bf16, tag=f"vbf{h}")
                nc.vector.tensor_copy(out=vbf, in_=vraw)

                # global pooled K/V
                kgT = persist.tile([D, KLEN], f32r, tag=f"kgT{h}", name=f"kgT{h}")
                kgf = small.tile([D, KLEN], f32, tag="kgf")  # f32 scratch
                nc.sync.dma_start(out=kgf[:, 0:1], in_=zk_d.ap()[h])
                nc.vector.tensor_reduce(
                    out=kgf[:, 1:KLEN],
                    in_=KT[h].bitcast(f32).rearrange("p (g s) -> p g s", s=S),
                    axis=mybir.AxisListType.X,
                    op=ADD,
                )
                nc.vector.tensor_copy(out=kgT[:, 0:1], in_=kgf[:, 0:1])
                nc.vector.tensor_scalar_mul(
                    out=kgT[:, 1:KLEN], in0=kgf[:, 1:KLEN], scalar1=1.0 / S
                )
                # vg pooling via PE: vgTp[:, g+1] = V_g^T @ (1/S)
                vgTp = scores_p.tile([D, KLEN], f32, tag="s")
                for g in range(NT):
                    nc.tensor.matmul(
                        out=vgTp[:, g + 1 : g + 2],
                        lhsT=vbf[:, g, :],
                        rhs=poolcol,
                        start=(g == 0),
                        stop=(g == NT - 1),
                    )
                vgT = small.tile([D, KLEN], f32, tag="vgT")
                nc.sync.dma_start(out=vgT[:, 0:1], in_=zv_d.ap()[h])
                nc.vector.tensor_copy(out=vgT[:, 1:KLEN], in_=vgTp[:, 1:KLEN])
                # transpose -> Vg token-major [KLEN, D] bf16
                vgp = scores_p.tile([KLEN, 128], f32, tag="s")
                nc.tensor.transpose(out=vgp, in_=vgT, identity=ident)
                Vg = persist.tile([KLEN, 128], bf16, tag=f"Vg{h}")
                nc.vector.tensor_copy(out=Vg, in_=vgp)

                # blocks: global chunks then local block
                for p in range(PNUM):
                    og_norm = {}
                    # ---- global chunks 2p, 2p+1
                    for c in (2 * p, 2 * p + 1):
                        rows = min(KLEN, 4 * c + 4)
                        qs = slice(c * CHUNK, (c + 1) * CHUNK)
                        sg = scores_p.tile([128, 1024], f32, tag="s")
                        nc.tensor.matmul(
                            out=sg[0:rows, 0:CHUNK],
                            lhsT=kgT[:, 0:rows],
                            rhs=QT[h][:, qs],
                            start=True,
                            stop=False,
                        )
                        nc.tensor.matmul(
                            out=sg[0:rows, 0:CHUNK],
                            lhsT=gB[:, KLEN * c : KLEN * c + rows],
                            rhs=gC,
                            start=False,
                            stop=True,
                        )
                        eg = expp.tile([128, 1024], bf16, tag="e")
                        nc.scalar.activation(
                            out=eg[0:rows, 0:CHUNK],
                            in_=sg[0:rows, 0:CHUNK],
                            func=EXP,
                            scale=SCALE,
                        )
                        gs = acc_p.tile([128, 1024], f32, tag="sum")
                        nc.tensor.matmul(
                            out=gs[:, 0:CHUNK],
                            lhsT=ones_bf[0:rows, :],
                            rhs=eg[0:rows, 0:CHUNK],
                            start=True,
                            stop=True,
                        )
                        go = acc_p.tile([128, 1024], f32, tag="o")
                        nc.tensor.matmul(
                            out=go[:, 0:CHUNK],
                            lhsT=Vg[0:rows, :],
                            rhs=eg[0:rows, 0:CHUNK],
                            start=True,
                            stop=True,
                        )
                        rg = combine.tile([128, CHUNK], f32, tag="rg")
                        nc.vector.reciprocal(out=rg, in_=gs[:, 0:CHUNK])
                        ogn = combine.tile([128, CHUNK], f32, tag="ogn")
                        nc.vector.tensor_tensor(
                            out=ogn, in0=go[:, 0:CHUNK], in1=rg, op=MUL
                        )
                        og_norm[c] = ogn

                    # ---- local block p
                    q0 = p * L
                    ms = list(range(1, 9)) if p == 0 else list(range(0, 9))
                    expt = {}
                    sums = acc_p.tile([128, 1024], f32, tag="sum")
                    sum_started = [False, False]
                    sum_last_m = {
                        reg: max(
                            m
                            for m in ms
                            if (0 if m == 0 else 128 * (m - 1)) < 512 * (reg + 1)
                        )
                        for reg in (0, 1)
                    }
                    for m in ms:
                        start_m = 0 if m == 0 else 128 * (m - 1)
                        kcol = q0 - 128 + 128 * m  # k-token start (abs)
                        st = scores_p.tile([128, 1024], f32, tag="s")
                        # QK^T: per <=512-col PSUM region
                        for r0 in range(start_m - start_m % 512, 1024, 512):
                            c_lo = max(start_m, r0)
                            c_hi = r0 + 512
                            is_diag_reg = m >= 1 and start_m >= r0
                            nc.tensor.matmul(
                                out=st[:, c_lo:c_hi],
                                lhsT=KT[h][:, kcol : kcol + 128],
                                rhs=QT[h][:, q0 + c_lo : q0 + c_hi],
                                start=True,
                                stop=not is_diag_reg,
                            )
                            if is_diag_reg:
                                nc.tensor.matmul(
                                    out=st[:, start_m : start_m + 128],
                                    lhsT=mB,
                                    rhs=mC,
                                    start=False,
                                    stop=True,
                                )
                        et = expp.tile([128, 1024], bf16, tag="e")
                        nc.scalar.activation(
                            out=et[:, start_m:1024],
                            in_=st[:, start_m:1024],
                            func=EXP,
                            scale=SCALE,
                        )
                        expt[m] = et
                        # accumulate column sums: sums[:, c] += sum_k et[k, c]
                        for reg in (0, 1):
                            c_lo = max(start_m, reg * 512)
                            c_hi = (reg + 1) * 512
                            if c_lo >= c_hi:
                                continue
                            nc.tensor.matmul(
                                out=sums[:, c_lo:c_hi],
                                lhsT=ones_bf,
                                rhs=et[:, c_lo:c_hi],
                                start=not sum_started[reg],
                                stop=(m == sum_last_m[reg]),
                            )
                            sum_started[reg] = True
                    # PV: O^T accumulation per 512-col region
                    ot = acc_p.tile([128, 1024], f32, tag="o")
                    for reg in (0, 1):
                        valid_ms = [
                            m
                            for m in ms
                            if (0 if m == 0 else 128 * (m - 1)) < 512 * (reg + 1)
                        ]
                        for i, m in enumerate(valid_ms):
                            start_m = 0 if m == 0 else 128 * (m - 1)
                            c_lo = max(start_m, reg * 512)
                            c_hi = (reg + 1) * 512
                            vidx = 8 * p - 1 + m
                            nc.tensor.matmul(
                                out=ot[:, c_lo:c_hi],
                                lhsT=vbf[:, vidx, :],
                                rhs=expt[m][:, c_lo:c_hi],
                                start=(i == 0),
                                stop=(m == valid_ms[-1]),
                            )
                    # normalize + combine with global, write out
                    for reg in (0, 1):
                        c = 2 * p + reg
                        cols = slice(reg * 512, (reg + 1) * 512)
                        rl = combine.tile([128, CHUNK], f32, tag="rl")
                        nc.vector.reciprocal(out=rl, in_=sums[:, cols])
                        tl = combine.tile([128, CHUNK], f32, tag="tl")
                        nc.vector.tensor_tensor(
                            out=tl, in0=ot[:, cols], in1=rl, op=MUL
                        )
                        fin = combine.tile([128, CHUNK], f32, tag="fin")
                        nc.gpsimd.tensor_tensor(
                            out=fin, in0=tl, in1=og_norm[c], op=ADD
                        )
                        nc.sync.dma_start(
                            out=out_d.ap()[h, :, c * CHUNK : (c + 1) * CHUNK],
                            in_=fin,
                        )
    return nc


def _get_program():
    if "nc" not in _CACHE:
        _CACHE["nc"] = _build_program()
        _CACHE["consts"] = _host_constants()
    return _CACHE["nc"], _CACHE["consts"]


# ---------------------------------------------------------------- entry point
def kernel(q, k, v, zero_k, zero_v):
    nc, consts = _get_program()
    from concourse.bass_utils import run_bass_kernel_spmd

    q4 = np.asarray(q, dtype=np.float32).reshape(T, H, D)
    k4 = np.asarray(k, dtype=np.float32).reshape(T, H, D)
    v4 = np.asarray(v, dtype=np.float32).reshape(T, H, D)
    zk = np.asarray(zero_k, dtype=np.float32).reshape(H, D)
    zv = np.asarray(zero_v, dtype=np.float32).reshape(H, D)

    in_maps = []
    for core in range(NCORES):
        hs = slice(HPC * core, HPC * (core + 1))
        in_maps.append(
            {
                "qT": np.ascontiguousarray(q4[:, hs].transpose(1, 2, 0)),
                "kT": np.ascontiguousarray(k4[:, hs].transpose(1, 2, 0)),
                "v": np.ascontiguousarray(v4[:, hs].transpose(1, 0, 2)),
                "zk": np.ascontiguousarray(zk[hs])[:, :, None],
                "zv": np.ascontiguousarray(zv[hs])[:, :, None],
                **consts,
            }
        )

    res = run_bass_kernel_spmd(nc, in_maps, core_ids=list(range(NCORES)))
    # outT per core: [HPC, D, T] -> out[t, 0, (2*core+h)*D + d]
    arr = np.stack([res.results[i]["outT"] for i in range(NCORES)])  # [8, 2, D, T]
    out = arr.transpose(3, 0, 1, 2).reshape(T, 1, H * D)
    return np.ascontiguousarray(out.astype(np.float32))
